# revision 1
# baseline (speedup 1.0000x reference)
"""Trainium2 Bass kernel for nn_Basic_MPNN (gnn_message_passing).

Math (per batch b):
  m1 = node @ W1 + b1                  [N, MID]   (receiver side, axis 2)
  m2 = node @ W2 + b2                  [N, MID]   (sender side, axis 1)
  me = edge @ We + be                  [N, N, MID]
  mg = graph @ Wg + bg                 [MID]
  msgs[j,i,:] = m1[i] + m2[j] + me[j,i] + mg
  M[i,:] = max_j where(adj[j,i], msgs[j,i,:], -1e6)
  out = relu(node @ Wo1 + bo1 + M @ Wo2 + bo2)

Sharding: 8 cores = (4 batches) x (2 receiver halves of 256).

Per-core device algorithm (roofline = streaming the 64 MiB edge slice):
  cT[mid,i] = (m1[i] + mg + b1+be+bg)^T computed once (fp32).
  For each sender j and receiver block: DMA edge tile [128 i, 128 d]
  (fp32->fp16 cast in the DMA), PE-transpose to [d, i]; per 4-j batch one
  N=512 fp16 matmul with stationary We producing meT slots [mid, j*128+i]
  in PSUM; then per j a rank-2 matmul accumulates
    adj01[j,i] * m2[j,mid] + (1-adj01[j,i]) * (-60000)
  which applies mask and sender term exactly (products with the 0/1
  gate are exact; no large-constant rounding touches live values).
  DVE reduce_max folds 8 slots at a time, then two more 8-way levels,
  all in [mid, i] layout.
  Finalize: M = max(Mraw + cT, -1e6); out = relu(noderT.T@Wo1 + M.T'@Wo2 + b).

Rank-2 row-group placement: all rank-2 matmuls of sender-group g share PE
row-group k = g // 16 (two adjacent row-grouped matmuls with *different*
tile_position inside an open PSUM accumulation group crash the HW --
verified experimentally; same tile_position back-to-back is fine, and a
full-K matmul between them is fine). The build asserts the final PE
schedule has no unsafe adjacency.
"""

import os
import sys

for _p in (
    "/root/.axon_site",
    "/root/.axon_site/_ro/trn_rl_repo",
    "/root/.axon_site/_ro/pypackages",
    "/opt/trn_rl_repo",
    "/opt/pypackages",
):
    if os.path.isdir(_p) and _p not in sys.path:
        sys.path.append(_p)

import numpy as np  # noqa: E402

import concourse.bass as bass  # noqa: E402
import concourse.tile as tile  # noqa: E402
from concourse import bacc, masks, mybir  # noqa: E402
from concourse.bass_utils import run_bass_kernel_spmd  # noqa: E402

F32 = mybir.dt.float32
F16 = mybir.dt.float16
I32 = mybir.dt.int32

B, N, D, MID, OUT = 4, 512, 128, 128, 128
NCORES = 8
IH = N // 2  # receivers per core
JG = 8       # senders per j-group
NG = N // JG  # 64 j-groups
L2W = 8      # groups per level-2 reduce
MASK_NEG = -60000.0  # < any valid msg value; fp16-representable
BIG_NUMBER = 1.0e6


def _k_of_group(g):
    # row-group for sender-group g; constant across 16-group spans so
    # adjacent rank-2 matmuls share tile_position almost everywhere
    return g // 16


def _u_of_j(j):
    return j % 128


def _build_program(repeat=1):
    nc = bacc.Bacc(
        "TRN2", target_bir_lowering=False, debug=False, num_devices=NCORES
    )

    edge = nc.dram_tensor("edge", [N, IH, D], F32, kind="ExternalInput").ap()
    nodeT_d = nc.dram_tensor("nodeT", [D, N], F32, kind="ExternalInput").ap()
    noderT_d = nc.dram_tensor("noderT", [D, IH], F32, kind="ExternalInput").ap()
    graph = nc.dram_tensor("graph", [1, D], F32, kind="ExternalInput").ap()
    adj = nc.dram_tensor("adj", [N, IH], I32, kind="ExternalInput").ap()
    wpack_d = nc.dram_tensor("wpack", [D, 5 * MID], F32, kind="ExternalInput").ap()
    bpack_d = nc.dram_tensor("bpack", [1, 6 * MID], F32, kind="ExternalInput").ap()
    we_d = nc.dram_tensor("We", [D, MID], F32, kind="ExternalInput").ap()
    out_d = nc.dram_tensor("out", [IH, OUT], F32, kind="ExternalOutput").ap()

    with (
        tile.TileContext(nc) as tc,
        tc.tile_pool(name="persist", bufs=1) as pp,
        tc.tile_pool(name="setup_sb", bufs=1) as ssb,
        tc.tile_pool(name="accum", bufs=1) as accp,
        tc.tile_pool(name="edge", bufs=8) as ep,
        tc.tile_pool(name="tf", bufs=8) as tfp,
        tc.tile_pool(name="ps8", bufs=3, space="PSUM") as ps8p,
        tc.tile_pool(name="psT", bufs=2, space="PSUM") as psTp,
    ):
        if True:
            # ---------------- adjacency in rank-2 rhs layout ----------------
            # adjn[u, k, i] = adj[128k+u, i]
            adjn = ssb.tile([128, 4 * IH], I32)
            nc.sync.dma_start(
                adjn[:], adj.rearrange("(k u) i -> u k i", k=4)
            )
            a01_32 = ssb.tile([128, 4 * IH], F32)
            nc.vector.tensor_copy(a01_32[:], adjn[:])
            a01 = ssb.tile([128, 4 * IH], F16)
            nc.vector.tensor_copy(a01[:], a01_32[:])
            inv01 = ssb.tile([128, 4 * IH], F16)
            nc.vector.tensor_scalar(
                inv01[:], a01_32[:], -1.0, 1.0,
                op0=mybir.AluOpType.mult, op1=mybir.AluOpType.add,
            )
            # adjr2[32k+0, u*256 + ib*128 + il] = adj01[j, ib*128+il]
            adjr2 = pp.tile([128, 128 * IH], F16)
            for k in range(4):
                nc.sync.dma_start(
                    adjr2[32 * k:32 * k + 1, :], a01[:, k * IH:(k + 1) * IH]
                )
                nc.scalar.dma_start(
                    adjr2[32 * k + 1:32 * k + 2, :],
                    inv01[:, k * IH:(k + 1) * IH],
                )
            # ---------------- constants & weights ----------------
            ident16 = pp.tile([128, 128], F16)
            masks.make_identity(nc, ident16[:])
            ones32 = pp.tile([1, 256], F32)
            nc.vector.memset(ones32[:], 1.0)

            # node features first: they gate the m2 -> m2r2 chain
            nodeT = pp.tile([D, N], F32)
            nc.sync.dma_start(nodeT[:], nodeT_d[:, :])
            noderT = pp.tile([D, IH], F32)
            nc.scalar.dma_start(noderT[:], noderT_d[:, :])
            wpack = pp.tile([D, 5 * MID], F32)
            nc.sync.dma_start(wpack[:], wpack_d[:, :])
            bpack = pp.tile([1, 6 * MID], F32)
            nc.scalar.dma_start(bpack[:], bpack_d[:, :])
            wsb = {
                w: wpack[:, i * MID:(i + 1) * MID]
                for i, w in enumerate(("W2", "W1", "Wg", "Wo1", "Wo2"))
            }
            bsb = {
                b: bpack[:, i * MID:(i + 1) * MID]
                for i, b in enumerate(("b1", "b2", "be", "bg", "bo1", "bo2"))
            }
            we16 = pp.tile([D, MID], F16)
            nc.gpsimd.dma_start(we16[:], we_d[:, :])  # cast f32->f16

            # ---------------- m2 in rank-2 lhsT layout ----------------
            # m2r2[32k+0, u*128+mid] = m2[j, mid] (f16), j = 128k + u;
            # m2r2[32k+1, ...] = MASK_NEG
            m2r2 = pp.tile([128, 128 * MID], F16)
            neg_sb = ssb.tile([128, 512], F16)
            nc.vector.memset(neg_sb[:], MASK_NEG)
            m2f16 = ssb.tile([128, 4 * MID], F16)
            # nodeT columns j = 128k + u
            for k in range(4):
                ps_m2 = psTp.tile([128, MID], F32, tag="pT")
                nc.tensor.matmul(
                    ps_m2[:],
                    lhsT=nodeT[:, k * 128:(k + 1) * 128],
                    rhs=wsb["W2"], start=True, stop=False,
                )
                nc.tensor.matmul(
                    ps_m2[:], lhsT=ones32[:, 0:128], rhs=bsb["b2"],
                    start=False, stop=True,
                )
                nc.scalar.copy(m2f16[:, k * MID:(k + 1) * MID], ps_m2[:])
            for k in range(4):
                nc.sync.dma_start(
                    m2r2[32 * k:32 * k + 1, :],
                    m2f16[:, k * MID:(k + 1) * MID],
                )
                nc.scalar.dma_start(
                    m2r2[32 * k + 1:32 * k + 2, :], neg_sb[0:32, :]
                )


            # r = mg + b1 + be + bg ; bso = bo1 + bo2
            gT = ssb.tile([D, 1], F32)
            nc.sync.dma_start(gT[:], graph[0:1, :])
            ps_mg = psTp.tile([1, MID], F32, tag="pT")
            nc.tensor.matmul(ps_mg[:], lhsT=gT[:], rhs=wsb["Wg"], start=True, stop=True)
            r_sb = pp.tile([1, MID], F32)
            nc.scalar.copy(r_sb[:], ps_mg[:])
            nc.vector.tensor_add(r_sb[:], r_sb[:], bsb["b1"])
            nc.vector.tensor_add(r_sb[:], r_sb[:], bsb["be"])
            nc.vector.tensor_add(r_sb[:], r_sb[:], bsb["bg"])
            bso = pp.tile([1, MID], F32)
            nc.vector.tensor_add(bso[:], bsb["bo1"], bsb["bo2"])

            # ---------------- cT[mid, i] = (m1 + r)^T ----------------
            ps_cT = psTp.tile([128, IH], F32, name="ps_cT", tag="pT")
            nc.tensor.matmul(
                ps_cT[:], lhsT=wsb["W1"][:], rhs=noderT[:], start=True, stop=False
            )
            nc.tensor.matmul(
                ps_cT[:], lhsT=r_sb[:], rhs=ones32[:], start=False, stop=True
            )
            cT_sb = pp.tile([128, IH], F32)
            nc.scalar.copy(cT_sb[:], ps_cT[:])

        # ---------------- main streaming loop ----------------
        redbuf = [None, None]
        l2buf = [None, None]
        if True:
            for ib in range(2):
                redbuf[ib] = accp.tile([128, 2 * L2W * MID], F32, name=f"red{ib}")
                l2buf[ib] = accp.tile([128, (NG // L2W) * MID], F32, name=f"l2{ib}")

            if True:
                # Software pipeline: per unit (g, ib) emit the transposes and
                # PSUM->SBUF copies; the We-matmuls + rank-2 + reduce for a
                # unit are emitted one unit later so the PE never head-of-line
                # blocks on the Activation copy of its own transposes.
                def emit_mm_reduce(st):
                    g, ib, tfs = st
                    k = _k_of_group(g)
                    ps8 = ps8p.tile([128, JG * MID], F32, tag="ps8")
                    for half in range(2):
                        nc.tensor.matmul(
                            ps8[:, half * 512:(half + 1) * 512],
                            lhsT=we16[:], rhs=tfs[half][:],
                            start=True, stop=False,
                        )
                        for q in range(4):
                            jl = half * 4 + q
                            j = g * JG + jl
                            u = _u_of_j(j)
                            nc.tensor.matmul(
                                ps8[:, jl * MID:(jl + 1) * MID],
                                lhsT=m2r2[32 * k:32 * k + 2,
                                          u * 128:(u + 1) * 128],
                                rhs=adjr2[32 * k:32 * k + 2,
                                          u * 256 + ib * 128:u * 256 + ib * 128 + 128],
                                start=False, stop=(q == 3),
                                tile_position=(32 * k, 0),
                            )
                    slot = g % (2 * L2W)
                    nc.vector.tensor_reduce(
                        redbuf[ib][:, slot * MID:(slot + 1) * MID],
                        ps8[:].rearrange("p (s m) -> p m s", s=JG),
                        axis=mybir.AxisListType.X,
                        op=mybir.AluOpType.max,
                    )
                    if g % L2W == L2W - 1:
                        par = (g // L2W) % 2
                        nc.vector.tensor_reduce(
                            l2buf[ib][:, (g // L2W) * MID:(g // L2W + 1) * MID],
                            redbuf[ib][:, par * L2W * MID:(par + 1) * L2W * MID]
                            .rearrange("p (s m) -> p m s", s=L2W),
                            axis=mybir.AxisListType.X,
                            op=mybir.AluOpType.max,
                        )

                stash = []
                e_t = None
                for g in range(repeat * NG):
                    g = g % NG
                    e_t = ep.tile([128, JG * 2 * D], F16, tag="e")
                    nc.gpsimd.dma_start(
                        e_t[:],
                        edge[g * JG:(g + 1) * JG]
                        .rearrange("j (ib p) d -> p j ib d", p=128),
                    )
                    for ib in range(2):
                        tfs = []
                        for half in range(2):
                            pT = psTp.tile([128, 512], F16, tag="pT")
                            for q in range(4):
                                jl = half * 4 + q
                                nc.tensor.transpose(
                                    pT[:, q * 128:(q + 1) * 128],
                                    e_t[:, (jl * 2 + ib) * D:(jl * 2 + ib + 1) * D],
                                    ident16[:],
                                )
                            tf = tfp.tile([128, 512], F16, tag="tf")
                            nc.scalar.copy(tf[:], pT[:])
                            tfs.append(tf)
                        stash.append((g, ib, tfs))
                        if len(stash) > 1:
                            emit_mm_reduce(stash.pop(0))
                while stash:
                    emit_mm_reduce(stash.pop(0))

            # ---------------- finalize ----------------
            with (
                tc.tile_pool(name="fin_sb", bufs=2) as fsb,
            ):
                fps = psTp
                for ib in range(2):
                    mraw = fsb.tile([128, MID], F32, tag="mraw")
                    nc.vector.tensor_reduce(
                        mraw[:],
                        l2buf[ib][:].rearrange("p (s m) -> p m s", s=NG // L2W),
                        axis=mybir.AxisListType.X,
                        op=mybir.AluOpType.max,
                    )
                    # msgs^T [mid, i] = max(mraw + cT, -1e6)
                    msgs = fsb.tile([128, MID], F32, tag="msgs")
                    nc.vector.tensor_add(
                        msgs[:], mraw[:], cT_sb[:, ib * MID:(ib + 1) * MID]
                    )
                    nc.vector.tensor_scalar_max(msgs[:], msgs[:], -BIG_NUMBER)
                    ps_h = fps.tile([128, OUT], F32, tag="pT")
                    nc.tensor.matmul(
                        ps_h[:], lhsT=msgs[:], rhs=wsb["Wo2"],
                        start=True, stop=False,
                    )
                    nc.tensor.matmul(
                        ps_h[:], lhsT=noderT[:, ib * 128:(ib + 1) * 128],
                        rhs=wsb["Wo1"], start=False, stop=False,
                    )
                    nc.tensor.matmul(
                        ps_h[:], lhsT=ones32[:, 0:128], rhs=bso[:],
                        start=False, stop=True,
                    )
                    o_sb = fsb.tile([128, OUT], F32, tag="osb")
                    nc.scalar.activation(
                        o_sb[:], ps_h[:], mybir.ActivationFunctionType.Relu
                    )
                    nc.sync.dma_start(out_d[ib * 128:(ib + 1) * 128, :], o_sb[:])

    nc.finalize()
    _assert_safe_pe_schedule(nc)
    return nc


def _assert_safe_pe_schedule(nc):
    """No two adjacent sub-tile (row-grouped) matmuls with different
    tile_position in the final PE stream (HW crash pattern)."""
    prev = None
    for func in nc.m.functions:
        for block in func.blocks:
            for inst in block.instructions:
                if not isinstance(inst, mybir.InstMatmult):
                    continue
                rows = inst.tile_size[0] if inst.tile_size else 128
                sub = rows < 128
                cur = (sub, tuple(inst.tile_position or (0, 0)))
                if (
                    prev is not None
                    and prev[0] and sub
                    and prev[1] != cur[1]
                ):
                    raise AssertionError(
                        f"unsafe adjacent row-grouped matmuls: {prev} -> {cur}"
                    )
                prev = cur
    return True


_CACHED = {}


def _get_program():
    if "nc" not in _CACHED:
        _CACHED["nc"] = _build_program()
    return _CACHED["nc"]


def kernel(**inputs) -> np.ndarray:
    nc = _get_program()

    def f32(x):
        return np.ascontiguousarray(np.asarray(x, dtype=np.float32))

    node_fts = f32(inputs["node_fts"])
    edge_fts = f32(inputs["edge_fts"])
    graph_fts = f32(inputs["graph_fts"])
    adj_mat = np.ascontiguousarray(np.asarray(inputs["adj_mat"], dtype=np.int32))

    shared = {}
    shared["wpack"] = np.ascontiguousarray(np.concatenate(
        [f32(inputs[w]) for w in ("W2", "W1", "Wg", "Wo1", "Wo2")], axis=1
    ))
    shared["bpack"] = np.ascontiguousarray(np.concatenate(
        [f32(inputs[b]).reshape(1, MID)
         for b in ("b1", "b2", "be", "bg", "bo1", "bo2")], axis=1
    ))
    shared["We"] = f32(inputs["We"])

    in_maps = []
    for c in range(NCORES):
        b, ih = c // 2, c % 2
        sl = slice(ih * IH, (ih + 1) * IH)
        m = dict(shared)
        m["edge"] = np.ascontiguousarray(edge_fts[b, :, sl, :])
        m["nodeT"] = np.ascontiguousarray(node_fts[b].T)
        m["noderT"] = np.ascontiguousarray(node_fts[b, sl, :].T)
        m["graph"] = np.ascontiguousarray(graph_fts[b]).reshape(1, D)
        m["adj"] = np.ascontiguousarray(adj_mat[b, :, sl])
        in_maps.append(m)

    res = run_bass_kernel_spmd(nc, in_maps, list(range(NCORES)))

    out = np.empty((B, N, OUT), dtype=np.float32)
    for c in range(NCORES):
        b, ih = c // 2, c % 2
        out[b, ih * IH:(ih + 1) * IH, :] = res.results[c]["out"]
    return out



# revision 30
# speedup vs baseline: 1.6085x; 1.6085x over previous
"""Trainium2 Bass kernel for nn_Basic_MPNN (gnn_message_passing).

Math (per batch b):
  m1 = node @ W1 + b1                  [N, MID]   (receiver side, axis 2)
  m2 = node @ W2 + b2                  [N, MID]   (sender side, axis 1)
  me = edge @ We + be                  [N, N, MID]
  mg = graph @ Wg + bg                 [MID]
  msgs[j,i,:] = m1[i] + m2[j] + me[j,i] + mg
  M[i,:] = max_j where(adj[j,i], msgs[j,i,:], -1e6)
  out = relu(node @ Wo1 + bo1 + M @ Wo2 + bo2)

Sharding: 8 cores = (4 batches) x (2 receiver halves of 256).

Host prep (layout/cast only): the edge slice is pre-transposed and
pre-cast to fp8e4m3 as edgeT[d, j, i]; We ships as an fp8 hi/lo pair
(hi = fp8(We), lo = fp8(We - hi)) so the DoubleRow matmul reconstructs
near-f32 weights while streaming fp8; adj ships packed in the DoubleRow
rank-2 rhs layout ([adj01 | inv01] per (j, ib), duplicated on partition
rows 32k and 32k+1, k = j//128).

Per-core device algorithm:
  cT[mid,i] = (m1[i] + mg + b1+be+bg)^T computed once (fp32).
  Per sender-group g (8 j's) one DMA brings the fp8 edgeT tile; one
  4-bank PSUM unit holds BOTH receiver halves (ib=0,1). Per (ib, half)
  a DoubleRow matmul (lhsT = [We_hi|We_lo], rhs = the edge tile
  broadcast onto the Ko=2 dim with a stride-0 AP) writes
  me^T[mid, (jl,i)] at 0.5 cycles/column; per j a Ki=2 fp8 DoubleRow
  rank-2 accumulates
    adj01[j,i]*(m2_hi[j,mid]+m2_lo[j,mid]) + inv01[j,i]*MASK_NEG
  (0/1-gated products are exact in fp8; the m2 hi+lo split cancels the
  fp8 quantization of m2 to ~1e-3).
  The 8-slot max reduce (PSUM drain) works on whole pairs and is spread
  over three routes by group index: V (one DVE tensor_reduce straight
  from PSUM into both acc slots), AV (Act casts PSUM->f16 SBUF, DVE
  folds at 2x), AP (Act casts, Pool folds). Level-2 f16 folds (level 1
  on Pool, rest on DVE) fire every 8 g's; final fold + cT + clamp +
  output matmuls at the end.

Rank-2 row-group placement: all rank-2 matmuls of sender-group g share
PE row-group k = g // 16 (two adjacent row-grouped matmuls with
*different* tile_position inside an open PSUM accumulation group crash
the HW; same tile_position back-to-back is fine, and a full-K matmul
between them is fine). The build asserts the final PE schedule has no
unsafe adjacency.
"""

import os
import sys

for _p in (
    "/root/.axon_site",
    "/root/.axon_site/_ro/trn_rl_repo",
    "/root/.axon_site/_ro/pypackages",
    "/opt/trn_rl_repo",
    "/opt/pypackages",
):
    if os.path.isdir(_p) and _p not in sys.path:
        sys.path.append(_p)

import numpy as np  # noqa: E402

import concourse.bass as bass  # noqa: E402
import concourse.tile as tile  # noqa: E402
from concourse import bacc, mybir  # noqa: E402
from concourse.bass_utils import run_bass_kernel_spmd  # noqa: E402

F32 = mybir.dt.float32
F16 = mybir.dt.float16
F8 = mybir.dt.float8e4
NP_F8 = mybir.dt.np(F8)

B, N, D, MID, OUT = 4, 512, 128, 128, 128
NCORES = 8
IH = N // 2   # receivers per core
JG = 8        # senders per j-group
NG = N // JG  # 64 j-groups
NSLOT = 8     # accumulator slots per (ib, parity)
NWIN = NG // NSLOT  # 8 level-2 windows per ib
IBSHIFT = NSLOT // 2
MASK_NEG = -240.0  # < any valid msg; exact in fp8e4m3 (ml_dtypes e4m3 max is 240)
BIG_NUMBER = 1.0e6
PREFETCH = 4  # edge groups issued ahead of compute

MAX = mybir.AluOpType.max
DR = mybir.MatmulPerfMode.DoubleRow


# Drain route per unit index (unit = 2*g + ib), cycled. The GpSimd/Pool
# engine cannot execute any two-tensor ALU op on TRN2 (ISA check), so the
# drain runs entirely on DVE + Act:
#   'V'  = one DVE tensor_reduce straight from PSUM
#   'AV' = Act casts to f16 SBUF, DVE folds at 2x
ROUTE_PATTERN = (
    "V", "AV", "AV", "AV", "AV", "V", "AV", "AV", "AV", "AV",
    "V", "AV", "AV", "AV", "AV", "AV", "V", "AV", "AV", "AV", "AV",
)
ACC_NEG = -60000.0  # f16-representable filler for never-written acc slots


def _build_program():
    nc = bacc.Bacc(
        "TRN2", target_bir_lowering=False, debug=False, num_devices=NCORES
    )

    edgeT = nc.dram_tensor("edgeT", [D, N, IH], F8, kind="ExternalInput").ap()
    nodeT_d = nc.dram_tensor("nodeT", [D, N], F32, kind="ExternalInput").ap()
    noderT_d = nc.dram_tensor("noderT", [D, IH], F32, kind="ExternalInput").ap()
    graph = nc.dram_tensor("graph", [1, D], F32, kind="ExternalInput").ap()
    # DoubleRow rank-2 rhs rows: adjw[k, u*512 + ib*256 + 0:128] = adj01,
    # [...+128:256] = inv01, for j = 128k + u. Loaded onto partitions 32k
    # and 32k+1 (the Ki=2 contraction reads both).
    adjw_d = nc.dram_tensor("adjw", [8, 128 * 512], F8, kind="ExternalInput").ap()
    # DoubleRow lhsT template rows: row 2k (-> partition 32k) has MASK_NEG at
    # u*256+128:256, row 2k+1 (-> partition 32k+1) is zeros; m2_hi/m2_lo are
    # filled into the u*256+0:128 blocks on device.
    m2t_d = nc.dram_tensor("m2t", [8, 128 * 256], F8, kind="ExternalInput").ap()
    wpack_d = nc.dram_tensor("wpack", [D, 5 * MID], F32, kind="ExternalInput").ap()
    bpack_d = nc.dram_tensor("bpack", [1, 6 * MID], F32, kind="ExternalInput").ap()
    # [We_hi | We_lo] fp8 per partition row d
    we8_d = nc.dram_tensor("We8", [D, 2 * MID], F8, kind="ExternalInput").ap()
    out_d = nc.dram_tensor("out", [IH, OUT], F32, kind="ExternalOutput").ap()

    with (
        tile.TileContext(nc) as tc,
        tc.tile_pool(name="persist", bufs=1) as pp,
        tc.tile_pool(name="setup_sb", bufs=1) as ssb,
        tc.tile_pool(name="accum", bufs=1) as accp,
        tc.tile_pool(name="edge", bufs=6) as ep,
        tc.tile_pool(name="tv", bufs=6) as tvp,
        tc.tile_pool(name="ta", bufs=6) as tap,
        tc.tile_pool(name="tf", bufs=6) as tfp,
        tc.tile_pool(name="ps8", bufs=4, space="PSUM") as ps8p,
    ):
        # ---------------- critical-path DMAs first ----------------
        # HWDGE issue slots are ~625 ns each and shared; order matters.
        nodeT = pp.tile([D, N], F32)
        nc.sync.dma_start(nodeT[:], nodeT_d[:, :])
        wpack = pp.tile([D, 5 * MID], F32)
        nc.sync.dma_start(wpack[:], wpack_d[:, :])
        we8 = pp.tile([D, 2 * MID], F8)
        nc.sync.dma_start(we8[:], we8_d[:, :])

        # ---------------- edge prefetch ----------------
        ej = edgeT.rearrange("d (g j) i -> d g j i", j=JG)
        et_tiles = {}

        def issue_edge(g):
            et = ep.tile([128, JG * IH], F8, tag="e")
            nc.sync.dma_start(et[:], ej[:, g])
            et_tiles[g] = et

        for g in range(PREFETCH):
            issue_edge(g)

        # rank-2 operands: one 2-partition DMA per row-group k (host ships
        # adj duplicated onto both contraction rows)
        adjsb = pp.tile([128, 128 * 512], F8)
        m2dr = pp.tile([128, 128 * 256], F8)
        nc.sync.dma_start(m2dr[0:2, :], m2t_d[0:2, :])
        nc.sync.dma_start(adjsb[0:2, :], adjw_d[0:2, :])
        for k in range(1, 4):
            nc.sync.dma_start(
                m2dr[32 * k:32 * k + 2, :], m2t_d[2 * k:2 * k + 2, :])
            nc.sync.dma_start(
                adjsb[32 * k:32 * k + 2, :], adjw_d[2 * k:2 * k + 2, :])

        noderT = pp.tile([D, IH], F32)
        nc.gpsimd.dma_start(noderT[:], noderT_d[:, :])
        bpack = pp.tile([1, 6 * MID], F32)
        nc.gpsimd.dma_start(bpack[:], bpack_d[:, :])

        wsb = {
            w: wpack[:, i * MID:(i + 1) * MID]
            for i, w in enumerate(("W2", "W1", "Wg", "Wo1", "Wo2"))
        }
        bsb = {
            b: bpack[:, i * MID:(i + 1) * MID]
            for i, b in enumerate(("b1", "b2", "be", "bg", "bo1", "bo2"))
        }

        ones32 = pp.tile([1, 256], F32)
        nc.vector.memset(ones32[:], 1.0)

        # ---------------- m2 -> fp8 hi/lo DoubleRow lhsT rows ----------------
        # all four k-blocks in one borrowed PSUM tile, one wide cast chain
        ps_m2_t = ps8p.tile([128, JG * MID], F32, tag="ps8")
        for k in range(4):
            ps_k = ps_m2_t[:, k * MID:(k + 1) * MID]
            nc.tensor.matmul(
                ps_k,
                lhsT=nodeT[:, k * 128:(k + 1) * 128],
                rhs=wsb["W2"], start=True, stop=False,
            )
            nc.tensor.matmul(
                ps_k, lhsT=ones32[:, 0:128], rhs=bsb["b2"],
                start=False, stop=True,
            )
        m2f32 = ssb.tile([128, 4 * MID], F32)
        nc.scalar.copy(m2f32[:], ps_m2_t[:, 0:4 * MID])
        m2h8 = ssb.tile([128, 4 * MID], F8)
        nc.vector.tensor_copy(m2h8[:], m2f32[:])
        m2hup = ssb.tile([128, 4 * MID], F32)
        nc.vector.tensor_copy(m2hup[:], m2h8[:])
        m2l8 = ssb.tile([128, 4 * MID], F8)
        nc.vector.tensor_tensor(
            m2l8[:], m2f32[:], m2hup[:], op=mybir.AluOpType.subtract
        )
        for k in range(4):
            # m2dr[32k, u*256 + 0:128] = m2_hi[128k+u, :]; 32k+1 gets m2_lo
            for row, src in ((0, m2h8), (1, m2l8)):
                dst = m2dr[32 * k + row:32 * k + row + 1, :].rearrange(
                    "p (u c) -> p u c", c=256
                )[:, :, 0:128]
                nc.scalar.dma_start(dst, src[:, k * MID:(k + 1) * MID])

        # r = mg + b1 + be + bg ; bso = bo1 + bo2 ; cT[mid,i] = (m1 + r)^T
        gT = ssb.tile([D, 1], F32)
        nc.gpsimd.dma_start(gT[:], graph[0:1, :])
        ps_fin_t = ps8p.tile([128, JG * MID], F32, tag="ps8")
        ps_mg = ps_fin_t[0:1, 0:MID]
        nc.tensor.matmul(ps_mg, lhsT=gT[:], rhs=wsb["Wg"], start=True, stop=True)
        r_sb = pp.tile([1, MID], F32)
        nc.scalar.copy(r_sb[:], ps_mg)
        nc.vector.tensor_add(r_sb[:], r_sb[:], bsb["b1"])
        nc.vector.tensor_add(r_sb[:], r_sb[:], bsb["be"])
        nc.vector.tensor_add(r_sb[:], r_sb[:], bsb["bg"])
        bso = pp.tile([1, MID], F32)
        nc.vector.tensor_add(bso[:], bsb["bo1"], bsb["bo2"])

        ps_cT = ps_fin_t[:, MID:MID + IH]
        nc.tensor.matmul(
            ps_cT, lhsT=wsb["W1"][:], rhs=noderT[:], start=True, stop=False
        )
        nc.tensor.matmul(
            ps_cT, lhsT=r_sb[:], rhs=ones32[:], start=False, stop=True
        )
        cT_sb = pp.tile([128, IH], F32)
        nc.scalar.copy(cT_sb[:], ps_cT)

        # ---------------- accumulators ----------------
        # acc[ib]: [128, 2 parities * NSLOT slots * 128] f16.
        # Slot/parity indexing is shifted by IBSHIFT g's for ib=1 so the two
        # ibs' level-2 folds land on different g's (staggered bursts). Slots
        # a shifted window never writes are pre-filled with ACC_NEG; re-
        # folding stale slots from an older window is harmless under max.
        acc = [accp.tile([128, 2 * NSLOT * 128], F16, name=f"acc{ib}")
               for ib in range(2)]
        l2buf = [accp.tile([128, (NWIN + 1) * 128], F16, name=f"l2{ib}")
                 for ib in range(2)]
        nc.vector.memset(acc[1][:, 0:IBSHIFT * 128], ACC_NEG)
        nc.vector.memset(l2buf[0][:, NWIN * 128:(NWIN + 1) * 128], ACC_NEG)

        def gshift(g, ib):
            return g + IBSHIFT * ib

        def acc_dst(g, ib):
            gs = gshift(g, ib)
            slot = gs % NSLOT
            par = (gs // NSLOT) % 2
            lo = (par * NSLOT + slot) * 128
            return acc[ib][:, lo:lo + 128]

        def emit_release(ps8, g, ib, route):
            """The op that frees the PSUM unit: a single DVE reduce (V), or
            the Act cast copy (AV/AP). Returns the f16 tile the deferred
            folds read (None when already folded)."""
            if route == "V":
                nc.vector.tensor_reduce(
                    acc_dst(g, ib),
                    ps8.rearrange("p (s m) -> p m s", s=JG),
                    axis=mybir.AxisListType.X,
                    op=MAX,
                )
                return None
            c1 = tap.tile([128, 1024], F16, tag="ta")
            nc.scalar.copy(c1[:], ps8)
            return c1

        def emit_folds(st):
            g, ib, route, t = st
            if t is not None:
                dst = acc_dst(g, ib)
                eng = nc.vector
                t1 = tvp.tile([128, 512], F16, tag="tv")
                eng.tensor_tensor(t1[:], t[:, 0:512], t[:, 512:1024], op=MAX)
                t2 = tfp.tile([128, 256], F16, tag="tf")
                eng.tensor_tensor(t2[:], t1[:, 0:256], t1[:, 256:512], op=MAX)
                eng.tensor_tensor(dst, t2[:, 0:128], t2[:, 128:256], op=MAX)
            gs = gshift(g, ib)
            if gs % NSLOT == NSLOT - 1:
                l2fold(gs, ib)

        def l2fold(gs, ib):
            # fold the just-completed parity's NSLOT slots -> l2buf slot
            par = (gs // NSLOT) % 2
            base = par * NSLOT * 128
            src = acc[ib][:, base:base + NSLOT * 128]
            t1 = tvp.tile([128, 512], F16, tag="tvl")
            nc.vector.tensor_tensor(t1[:], src[:, 0:512], src[:, 512:1024],
                                    op=MAX)
            t3 = tfp.tile([128, 256], F16, tag="tf")
            nc.vector.tensor_tensor(t3[:], t1[:, 0:256], t1[:, 256:512],
                                    op=MAX)
            lslot = gs // NSLOT
            nc.vector.tensor_tensor(
                l2buf[ib][:, lslot * 128:(lslot + 1) * 128],
                t3[:, 0:128], t3[:, 128:256], op=MAX,
            )

        # ---------------- main streaming loop ----------------
        stash = []
        for g in range(NG):
            if g + PREFETCH < NG:
                issue_edge(g + PREFETCH)
            k = (g * JG) // 128
            et = et_tiles.pop(g)
            etv = et[:].rearrange("p (j i) -> p j i", j=JG)
            for ib in range(2):
                ps8_t = ps8p.tile([128, JG * MID], F32, tag="ps8")
                ps8 = ps8_t[:]
                for half in range(2):
                    rhs = (
                        etv[:, 4 * half:4 * half + 4, ib * 128:(ib + 1) * 128]
                        .unsqueeze(1)
                        .broadcast_to([128, 2, 4, 128])
                    )
                    nc.tensor.matmul(
                        ps8[:, half * 512:(half + 1) * 512],
                        lhsT=we8[:].rearrange("p (two f) -> p two f", two=2),
                        rhs=rhs,
                        start=True, stop=False,
                        perf_mode=DR,
                    )
                    for q in range(4):
                        jl = half * 4 + q
                        u = (g * JG + jl) % 128
                        lhsT = m2dr[32 * k:32 * k + 2,
                                    u * 256:(u + 1) * 256].rearrange(
                            "p (two f) -> p two f", two=2)
                        rhs2 = adjsb[32 * k:32 * k + 2,
                                     u * 512 + ib * 256:
                                     u * 512 + ib * 256 + 256].rearrange(
                            "p (two f) -> p two f", two=2)
                        nc.tensor.matmul(
                            ps8[:, jl * MID:(jl + 1) * MID],
                            lhsT=lhsT, rhs=rhs2,
                            start=False, stop=(q == 3),
                            perf_mode=DR,
                            tile_position=(32 * k, 0),
                        )
                unit = 2 * g + ib
                route = ROUTE_PATTERN[unit % len(ROUTE_PATTERN)]
                if g >= NG - 2:
                    route = "V"  # keep the tail off the slow Pool folds
                t = emit_release(ps8, g, ib, route)
                stash.append((g, ib, route, t))
                if len(stash) > 2:
                    emit_folds(stash.pop(0))
        while stash:
            emit_folds(stash.pop(0))
        # ib=1 tail window (shifted indexing): fold parity-0 slots holding
        # the last IBSHIFT g's plus stale-but-max-safe leftovers -> l2 slot 8
        l2fold(NG + NSLOT - 1, 1)

        # ---------------- finalize ----------------
        with tc.tile_pool(name="fin_sb", bufs=2) as fsb:
            o_sb = accp.tile([128, 2 * OUT], F32, name="o_sb")
            for ib in range(2):
                eng = nc.vector
                u1 = fsb.tile([128, 512], F16, tag="u1")
                eng.tensor_tensor(
                    u1[:], l2buf[ib][:, 0:512], l2buf[ib][:, 512:1024], op=MAX
                )
                u2 = fsb.tile([128, 256], F16, tag="u2")
                eng.tensor_tensor(u2[:], u1[:, 0:256], u1[:, 256:512], op=MAX)
                u3 = fsb.tile([128, MID], F16, tag="u3")
                eng.tensor_tensor(u3[:], u2[:, 0:128], u2[:, 128:256], op=MAX)
                mraw = fsb.tile([128, MID], F16, tag="mraw")
                eng.tensor_tensor(
                    mraw[:], u3[:], l2buf[ib][:, 1024:1152], op=MAX
                )
                # msgs^T [mid, i] = max(mraw + cT, -1e6)
                msgs = fsb.tile([128, MID], F32, tag="msgs")
                eng.tensor_tensor(
                    msgs[:], mraw[:], cT_sb[:, ib * MID:(ib + 1) * MID],
                    op=mybir.AluOpType.add,
                )
                eng.tensor_scalar_max(msgs[:], msgs[:], -BIG_NUMBER)
                ps_h_t = ps8p.tile([128, JG * MID], F32, tag="ps8")
                ps_h = ps_h_t[:, 0:OUT]
                nc.tensor.matmul(
                    ps_h, lhsT=msgs[:], rhs=wsb["Wo2"],
                    start=True, stop=False,
                )
                nc.tensor.matmul(
                    ps_h, lhsT=noderT[:, ib * 128:(ib + 1) * 128],
                    rhs=wsb["Wo1"], start=False, stop=False,
                )
                nc.tensor.matmul(
                    ps_h, lhsT=ones32[:, 0:128], rhs=bso[:],
                    start=False, stop=True,
                )
                nc.scalar.activation(
                    o_sb[:, ib * OUT:(ib + 1) * OUT], ps_h,
                    mybir.ActivationFunctionType.Relu,
                )
            nc.sync.dma_start(
                out_d.rearrange("(ib p) o -> p ib o", ib=2), o_sb[:]
            )

    nc.finalize()
    _assert_safe_pe_schedule(nc)
    return nc


def _assert_safe_pe_schedule(nc):
    """No two adjacent sub-tile (row-grouped) matmuls with different
    tile_position in the final PE stream (HW crash pattern)."""
    prev = None
    for func in nc.m.functions:
        for block in func.blocks:
            for inst in block.instructions:
                if not isinstance(inst, mybir.InstMatmult):
                    continue
                rows = inst.tile_size[0] if inst.tile_size else 128
                sub = rows < 128
                cur = (sub, tuple(inst.tile_position or (0, 0)))
                if (
                    prev is not None
                    and prev[0] and sub
                    and prev[1] != cur[1]
                ):
                    raise AssertionError(
                        f"unsafe adjacent row-grouped matmuls: {prev} -> {cur}"
                    )
                prev = cur
    return True


_CACHED = {}


def _get_program():
    if "nc" not in _CACHED:
        _CACHED["nc"] = _build_program()
    return _CACHED["nc"]


def kernel(**inputs) -> np.ndarray:
    nc = _get_program()

    def f32(x):
        return np.ascontiguousarray(np.asarray(x, dtype=np.float32))

    node_fts = f32(inputs["node_fts"])
    edge_fts = np.asarray(inputs["edge_fts"], dtype=np.float32)
    graph_fts = f32(inputs["graph_fts"])
    adj_mat = np.asarray(inputs["adj_mat"], dtype=np.int32)

    shared = {}
    shared["wpack"] = np.ascontiguousarray(np.concatenate(
        [f32(inputs[w]) for w in ("W2", "W1", "Wg", "Wo1", "Wo2")], axis=1
    ))
    shared["bpack"] = np.ascontiguousarray(np.concatenate(
        [f32(inputs[b]).reshape(1, MID)
         for b in ("b1", "b2", "be", "bg", "bo1", "bo2")], axis=1
    ))
    we = f32(inputs["We"])
    we_hi = we.astype(NP_F8)
    we_lo = (we - we_hi.astype(np.float32)).astype(NP_F8)
    shared["We8"] = np.ascontiguousarray(
        np.concatenate([we_hi, we_lo], axis=1))

    # DoubleRow lhsT template: MASK_NEG in the second 128-block per u on the
    # hi row (even), zeros on the lo row (odd)
    m2t = np.zeros((4, 2, 128, 2, 128), dtype=NP_F8)
    m2t[:, 0, :, 1, :] = np.float32(MASK_NEG).astype(NP_F8)
    shared["m2t"] = np.ascontiguousarray(m2t.reshape(8, 128 * 256))

    in_maps = []
    for c in range(NCORES):
        b, ih = c // 2, c % 2
        sl = slice(ih * IH, (ih + 1) * IH)
        m = dict(shared)
        # edgeT[d, j, i] fp8
        m["edgeT"] = np.ascontiguousarray(
            edge_fts[b, :, sl, :].transpose(2, 0, 1).astype(NP_F8))
        m["nodeT"] = np.ascontiguousarray(node_fts[b].T)
        m["noderT"] = np.ascontiguousarray(node_fts[b, sl, :].T)
        m["graph"] = np.ascontiguousarray(graph_fts[b]).reshape(1, D)
        # adjw[k, u*512 + ib*256 + {0:128 adj01, 128:256 inv01}], j = 128k+u
        a01 = (adj_mat[b, :, sl] != 0)          # [512 j, 256 i]
        aw = np.empty((4, 128, 2, 2, 128), dtype=NP_F8)
        a4 = a01.reshape(4, 128, 2, 128)         # [k, u, ib, i]
        aw[:, :, :, 0, :] = a4.astype(NP_F8)
        aw[:, :, :, 1, :] = (~a4).astype(NP_F8)
        aw = aw.reshape(4, 1, 128 * 512)
        m["adjw"] = np.ascontiguousarray(
            np.broadcast_to(aw, (4, 2, 128 * 512)).reshape(8, 128 * 512))
        in_maps.append(m)

    res = run_bass_kernel_spmd(nc, in_maps, list(range(NCORES)))

    out = np.empty((B, N, OUT), dtype=np.float32)
    for c in range(NCORES):
        b, ih = c // 2, c % 2
        out[b, ih * IH:(ih + 1) * IH, :] = res.results[c]["out"]
    return out


# revision 36
# speedup vs baseline: 1.7238x; 1.0717x over previous
"""Trainium2 Bass kernel for nn_Basic_MPNN (gnn_message_passing).

Math (per batch b):
  m1 = node @ W1 + b1                  [N, MID]   (receiver side, axis 2)
  m2 = node @ W2 + b2                  [N, MID]   (sender side, axis 1)
  me = edge @ We + be                  [N, N, MID]
  mg = graph @ Wg + bg                 [MID]
  msgs[j,i,:] = m1[i] + m2[j] + me[j,i] + mg
  M[i,:] = max_j where(adj[j,i], msgs[j,i,:], -1e6)
  out = relu(node @ Wo1 + bo1 + M @ Wo2 + bo2)

Sharding: 8 cores = (4 batches) x (2 receiver halves of 256).

Host prep (layout/cast only): the edge slice is pre-transposed and
pre-cast to fp8e4m3 as edgeT[d, j, i]; We ships as an fp8 hi/lo pair
(hi = fp8(We), lo = fp8(We - hi)) so the DoubleRow matmul reconstructs
near-f32 weights while streaming fp8; adj ships packed in the DoubleRow
rank-2 rhs layout ([adj01 | inv01] per (j, ib), duplicated on partition
rows 32k and 32k+1, k = j//128).

Per-core device algorithm:
  cT[mid,i] = (m1[i] + mg + b1+be+bg)^T computed once (fp32).
  Per sender-group g (8 j's) one DMA brings the fp8 edgeT tile; one
  4-bank PSUM unit holds BOTH receiver halves (ib=0,1). Per (ib, half)
  a DoubleRow matmul (lhsT = [We_hi|We_lo], rhs = the edge tile
  broadcast onto the Ko=2 dim with a stride-0 AP) writes
  me^T[mid, (jl,i)] at 0.5 cycles/column; per j a Ki=2 fp8 DoubleRow
  rank-2 accumulates
    adj01[j,i]*(m2_hi[j,mid]+m2_lo[j,mid]) + inv01[j,i]*MASK_NEG
  (0/1-gated products are exact in fp8; the m2 hi+lo split cancels the
  fp8 quantization of m2 to ~1e-3).
  The 8-slot max reduce (PSUM drain) works on whole pairs and is spread
  over three routes by group index: V (one DVE tensor_reduce straight
  from PSUM into both acc slots), AV (Act casts PSUM->f16 SBUF, DVE
  folds at 2x), AP (Act casts, Pool folds). Level-2 f16 folds (level 1
  on Pool, rest on DVE) fire every 8 g's; final fold + cT + clamp +
  output matmuls at the end.

Rank-2 row-group placement: all rank-2 matmuls of sender-group g share
PE row-group k = g // 16 (two adjacent row-grouped matmuls with
*different* tile_position inside an open PSUM accumulation group crash
the HW; same tile_position back-to-back is fine, and a full-K matmul
between them is fine). The build asserts the final PE schedule has no
unsafe adjacency.
"""

import os
import sys

for _p in (
    "/root/.axon_site",
    "/root/.axon_site/_ro/trn_rl_repo",
    "/root/.axon_site/_ro/pypackages",
    "/opt/trn_rl_repo",
    "/opt/pypackages",
):
    if os.path.isdir(_p) and _p not in sys.path:
        sys.path.append(_p)

import numpy as np  # noqa: E402

import concourse.bass as bass  # noqa: E402
import concourse.tile as tile  # noqa: E402
from concourse import bacc, mybir  # noqa: E402
from concourse.bass_utils import run_bass_kernel_spmd  # noqa: E402

F32 = mybir.dt.float32
F16 = mybir.dt.float16
F8 = mybir.dt.float8e4
NP_F8 = mybir.dt.np(F8)

B, N, D, MID, OUT = 4, 512, 128, 128, 128
NCORES = 8
IH = N // 2   # receivers per core
JG = 8        # senders per j-group
NG = N // JG  # 64 j-groups
NSLOT = 8     # accumulator slots per (ib, parity)
NWIN = NG // NSLOT  # 8 level-2 windows per ib
IBSHIFT = NSLOT // 2
MASK_NEG = -240.0  # < any valid msg; exact in fp8e4m3 (ml_dtypes e4m3 max is 240)
BIG_NUMBER = 1.0e6
PREFETCH = 4  # edge groups issued ahead of compute

MAX = mybir.AluOpType.max
DR = mybir.MatmulPerfMode.DoubleRow


# Drain route per group g (both ib units take the same route). The
# GpSimd/Pool engine cannot execute any two-tensor ALU op on TRN2 (ISA
# check), so the drain runs entirely on DVE + Act:
#   'V'  = DVE tensor_reduce straight from PSUM, one per unit
#   'AV' = Act casts each unit into half of a shared f16 tile; DVE folds
#          both halves together at 2x once the pair is complete
ROUTE_PATTERN = (
    "V", "AV", "AV", "AV", "AV", "V", "AV", "AV", "AV",
)
ACC_NEG = -60000.0  # f16-representable filler for never-written acc slots


def _build_program():
    nc = bacc.Bacc(
        "TRN2", target_bir_lowering=False, debug=False, num_devices=NCORES
    )

    edgeT = nc.dram_tensor("edgeT", [D, N, IH], F8, kind="ExternalInput").ap()
    nodeT_d = nc.dram_tensor("nodeT", [D, N], F32, kind="ExternalInput").ap()
    noderT_d = nc.dram_tensor("noderT", [D, IH], F32, kind="ExternalInput").ap()
    graph = nc.dram_tensor("graph", [1, D], F32, kind="ExternalInput").ap()
    # DoubleRow rank-2 rhs rows: adjw[k, u*512 + ib*256 + 0:128] = adj01,
    # [...+128:256] = inv01, for j = 128k + u. Loaded onto partitions 32k
    # and 32k+1 (the Ki=2 contraction reads both).
    adjw_d = nc.dram_tensor("adjw", [8, 128 * 512], F8, kind="ExternalInput").ap()
    # DoubleRow lhsT template rows: row 2k (-> partition 32k) has MASK_NEG at
    # u*256+128:256, row 2k+1 (-> partition 32k+1) is zeros; m2_hi/m2_lo are
    # filled into the u*256+0:128 blocks on device.
    m2t_d = nc.dram_tensor("m2t", [8, 128 * 256], F8, kind="ExternalInput").ap()
    wpack_d = nc.dram_tensor("wpack", [D, 5 * MID], F32, kind="ExternalInput").ap()
    bpack_d = nc.dram_tensor("bpack", [1, 6 * MID], F32, kind="ExternalInput").ap()
    # [We_hi | We_lo] fp8 per partition row d
    we8_d = nc.dram_tensor("We8", [D, 2 * MID], F8, kind="ExternalInput").ap()
    out_d = nc.dram_tensor("out", [IH, OUT], F32, kind="ExternalOutput").ap()

    with (
        tile.TileContext(nc) as tc,
        tc.tile_pool(name="persist", bufs=1) as pp,
        tc.tile_pool(name="setup_sb", bufs=1) as ssb,
        tc.tile_pool(name="accum", bufs=1) as accp,
        tc.tile_pool(name="edge", bufs=6) as ep,
        tc.tile_pool(name="tv", bufs=6) as tvp,
        tc.tile_pool(name="ta", bufs=6) as tap,
        tc.tile_pool(name="tf", bufs=6) as tfp,
        tc.tile_pool(name="ps8", bufs=4, space="PSUM") as ps8p,
    ):
        # ---------------- critical-path DMAs first ----------------
        # HWDGE issue slots are ~625 ns each and shared; order matters.
        nodeT = pp.tile([D, N], F32)
        nc.sync.dma_start(nodeT[:, 0:128], nodeT_d[:, 0:128])
        wpack = pp.tile([D, 5 * MID], F32)
        nc.sync.dma_start(wpack[:, 0:MID], wpack_d[:, 0:MID])  # W2 first
        we8 = pp.tile([D, 2 * MID], F8)
        nc.sync.dma_start(we8[:], we8_d[:, :])

        # ---------------- edge prefetch ----------------
        ej = edgeT.rearrange("d (g j) i -> d g j i", j=JG)
        et_tiles = {}

        def issue_edge(g):
            et = ep.tile([128, JG * IH], F8, tag="e")
            nc.sync.dma_start(et[:], ej[:, g])
            et_tiles[g] = et

        issue_edge(0)

        # rank-2 operands: one 2-partition DMA per row-group k (host ships
        # adj duplicated onto both contraction rows); k=0 ahead of the
        # remaining edge prefetch, later k's and the rest of wpack behind
        adjsb = pp.tile([128, 128 * 512], F8)
        m2dr = pp.tile([128, 128 * 256], F8)
        nc.sync.dma_start(m2dr[0:2, :], m2t_d[0:2, :])
        nc.sync.dma_start(adjsb[0:2, :], adjw_d[0:2, :])
        nc.sync.dma_start(nodeT[:, 128:], nodeT_d[:, 128:])
        for g in range(1, PREFETCH):
            issue_edge(g)
        nc.sync.dma_start(wpack[:, MID:], wpack_d[:, MID:])
        for k in range(1, 4):
            nc.sync.dma_start(
                m2dr[32 * k:32 * k + 2, :], m2t_d[2 * k:2 * k + 2, :])
            nc.sync.dma_start(
                adjsb[32 * k:32 * k + 2, :], adjw_d[2 * k:2 * k + 2, :])

        noderT = pp.tile([D, IH], F32)
        nc.gpsimd.dma_start(noderT[:], noderT_d[:, :])
        bpack = pp.tile([1, 6 * MID], F32)
        nc.gpsimd.dma_start(bpack[:], bpack_d[:, :])

        wsb = {
            w: wpack[:, i * MID:(i + 1) * MID]
            for i, w in enumerate(("W2", "W1", "Wg", "Wo1", "Wo2"))
        }
        bsb = {
            b: bpack[:, i * MID:(i + 1) * MID]
            for i, b in enumerate(("b1", "b2", "be", "bg", "bo1", "bo2"))
        }

        ones32 = pp.tile([1, 256], F32)
        nc.vector.memset(ones32[:], 1.0)

        # ---------------- m2 -> fp8 hi/lo DoubleRow lhsT rows ----------------
        # all four k-blocks in one borrowed PSUM tile, one wide cast chain
        ps_m2_t = ps8p.tile([128, JG * MID], F32, tag="ps8")
        for k in range(4):
            ps_k = ps_m2_t[:, k * MID:(k + 1) * MID]
            nc.tensor.matmul(
                ps_k,
                lhsT=nodeT[:, k * 128:(k + 1) * 128],
                rhs=wsb["W2"], start=True, stop=False,
            )
            nc.tensor.matmul(
                ps_k, lhsT=ones32[:, 0:128], rhs=bsb["b2"],
                start=False, stop=True,
            )
        m2f32 = ssb.tile([128, 4 * MID], F32)
        nc.scalar.copy(m2f32[:], ps_m2_t[:, 0:4 * MID])
        m2h8 = ssb.tile([128, 4 * MID], F8)
        nc.vector.tensor_copy(m2h8[:], m2f32[:])
        m2hup = ssb.tile([128, 4 * MID], F32)
        nc.vector.tensor_copy(m2hup[:], m2h8[:])
        m2l8 = ssb.tile([128, 4 * MID], F8)
        nc.vector.tensor_tensor(
            m2l8[:], m2f32[:], m2hup[:], op=mybir.AluOpType.subtract
        )
        for k in range(4):
            # m2dr[32k, u*256 + 0:128] = m2_hi[128k+u, :]; 32k+1 gets m2_lo
            for row, src in ((0, m2h8), (1, m2l8)):
                dst = m2dr[32 * k + row:32 * k + row + 1, :].rearrange(
                    "p (u c) -> p u c", c=256
                )[:, :, 0:128]
                nc.scalar.dma_start(dst, src[:, k * MID:(k + 1) * MID])

        # r = mg + b1 + be + bg ; bso = bo1 + bo2 ; cT[mid,i] = (m1 + r)^T
        gT = ssb.tile([D, 1], F32)
        nc.gpsimd.dma_start(gT[:], graph[0:1, :])
        ps_fin_t = ps8p.tile([128, JG * MID], F32, tag="ps8")
        ps_mg = ps_fin_t[0:1, 0:MID]
        nc.tensor.matmul(ps_mg, lhsT=gT[:], rhs=wsb["Wg"], start=True, stop=True)
        r_sb = pp.tile([1, MID], F32)
        nc.scalar.copy(r_sb[:], ps_mg)
        nc.vector.tensor_add(r_sb[:], r_sb[:], bsb["b1"])
        nc.vector.tensor_add(r_sb[:], r_sb[:], bsb["be"])
        nc.vector.tensor_add(r_sb[:], r_sb[:], bsb["bg"])
        bso = pp.tile([1, MID], F32)
        nc.vector.tensor_add(bso[:], bsb["bo1"], bsb["bo2"])

        ps_cT = ps_fin_t[:, MID:MID + IH]
        nc.tensor.matmul(
            ps_cT, lhsT=wsb["W1"][:], rhs=noderT[:], start=True, stop=False
        )
        nc.tensor.matmul(
            ps_cT, lhsT=r_sb[:], rhs=ones32[:], start=False, stop=True
        )
        cT_sb = pp.tile([128, IH], F32)
        nc.scalar.copy(cT_sb[:], ps_cT)

        # h1pre[i, out] = node @ Wo1 + bo1 + bo2 (receiver slice), so the
        # finalize needs only the msgs @ Wo2 matmul
        h1pre = pp.tile([128, 2 * OUT], F32)
        ps_h1_t = ps8p.tile([128, JG * MID], F32, tag="ps8")
        for ib in range(2):
            ps_h1 = ps_h1_t[:, ib * 512:ib * 512 + OUT]
            nc.tensor.matmul(
                ps_h1, lhsT=noderT[:, ib * 128:(ib + 1) * 128],
                rhs=wsb["Wo1"], start=True, stop=False,
            )
            nc.tensor.matmul(
                ps_h1, lhsT=ones32[:, 0:128], rhs=bso[:],
                start=False, stop=True,
            )
            nc.scalar.copy(h1pre[:, ib * OUT:(ib + 1) * OUT], ps_h1)

        # ---------------- accumulators ----------------
        # acc: [128, (ib 2, parity 2, slot 8, 128)] f16
        acc = accp.tile([128, 2 * 2 * NSLOT * 128], F16, name="acc")
        accv = acc[:].rearrange("p (i c) -> p i c", i=2)
        l2buf = [accp.tile([128, NWIN * 128], F16, name=f"l2{ib}")
                 for ib in range(2)]

        def acc_dst(g, ib):
            off = ((g // NSLOT) % 2 * NSLOT + g % NSLOT) * 128
            return acc[:, ib * 2048 + off:ib * 2048 + off + 128]

        def acc_dst2(g):
            # [p, 2, 128] strided AP over both ibs' slots for group g
            off = ((g // NSLOT) % 2 * NSLOT + g % NSLOT) * 128
            return accv[:, :, off:off + 128]

        def emit_release(ps8, g, ib, route, c1):
            """The op that frees the PSUM unit: a single DVE reduce (V), or
            an Act cast copy into half of the pair's shared f16 tile."""
            if route == "V":
                nc.vector.tensor_reduce(
                    acc_dst(g, ib),
                    ps8.rearrange("p (s m) -> p m s", s=JG),
                    axis=mybir.AxisListType.X,
                    op=MAX,
                )
            else:
                nc.scalar.copy(c1[:, ib * 1024:(ib + 1) * 1024], ps8)

        def emit_folds(st):
            g, route, c1 = st
            if route != "V":
                tv = c1[:].rearrange("p (i c) -> p i c", i=2)
                t1 = tvp.tile([128, 1024], F16, tag="tv")
                t1v = t1[:].rearrange("p (i c) -> p i c", i=2)
                nc.vector.tensor_tensor(
                    t1v, tv[:, :, 0:512], tv[:, :, 512:1024], op=MAX)
                t2 = tfp.tile([128, 512], F16, tag="tf")
                t2v = t2[:].rearrange("p (i c) -> p i c", i=2)
                nc.vector.tensor_tensor(
                    t2v, t1v[:, :, 0:256], t1v[:, :, 256:512], op=MAX)
                nc.vector.tensor_tensor(
                    acc_dst2(g), t2v[:, :, 0:128], t2v[:, :, 128:256], op=MAX)
            if g % NSLOT == NSLOT - 1:
                l2fold(g, 0)
                pending_l2.append((g + 2, g, 1))
            while pending_l2 and pending_l2[0][0] <= g:
                _, gf, ibf = pending_l2.pop(0)
                l2fold(gf, ibf)

        def l2fold(g, ib):
            # fold the just-completed parity's NSLOT slots -> l2buf slot
            par = (g // NSLOT) % 2
            base = (ib * 2 + par) * NSLOT * 128
            src = acc[:, base:base + NSLOT * 128]
            t1 = tvp.tile([128, 512], F16, tag="tvl")
            nc.vector.tensor_tensor(t1[:], src[:, 0:512], src[:, 512:1024],
                                    op=MAX)
            t3 = tfp.tile([128, 256], F16, tag="tf")
            nc.vector.tensor_tensor(t3[:], t1[:, 0:256], t1[:, 256:512],
                                    op=MAX)
            lslot = g // NSLOT
            nc.vector.tensor_tensor(
                l2buf[ib][:, lslot * 128:(lslot + 1) * 128],
                t3[:, 0:128], t3[:, 128:256], op=MAX,
            )

        # ---------------- main streaming loop ----------------
        stash = []
        pending_l2 = []
        for g in range(NG):
            if g + PREFETCH < NG:
                issue_edge(g + PREFETCH)
            k = (g * JG) // 128
            et = et_tiles.pop(g)
            etv = et[:].rearrange("p (j i) -> p j i", j=JG)
            route = "V" if g == NG - 1 else ROUTE_PATTERN[g % len(ROUTE_PATTERN)]
            c1 = None
            if route != "V":
                c1 = tap.tile([128, 2048], F16, tag="ta")
            for ib in range(2):
                ps8_t = ps8p.tile([128, JG * MID], F32, tag="ps8")
                ps8 = ps8_t[:]
                for half in range(2):
                    rhs = (
                        etv[:, 4 * half:4 * half + 4, ib * 128:(ib + 1) * 128]
                        .unsqueeze(1)
                        .broadcast_to([128, 2, 4, 128])
                    )
                    nc.tensor.matmul(
                        ps8[:, half * 512:(half + 1) * 512],
                        lhsT=we8[:].rearrange("p (two f) -> p two f", two=2),
                        rhs=rhs,
                        start=True, stop=False,
                        perf_mode=DR,
                    )
                    for q in range(4):
                        jl = half * 4 + q
                        u = (g * JG + jl) % 128
                        lhsT = m2dr[32 * k:32 * k + 2,
                                    u * 256:(u + 1) * 256].rearrange(
                            "p (two f) -> p two f", two=2)
                        rhs2 = adjsb[32 * k:32 * k + 2,
                                     u * 512 + ib * 256:
                                     u * 512 + ib * 256 + 256].rearrange(
                            "p (two f) -> p two f", two=2)
                        nc.tensor.matmul(
                            ps8[:, jl * MID:(jl + 1) * MID],
                            lhsT=lhsT, rhs=rhs2,
                            start=False, stop=(q == 3),
                            perf_mode=DR,
                            tile_position=(32 * k, 0),
                        )
                emit_release(ps8, g, ib, route, c1)
            stash.append((g, route, c1))
            if len(stash) > 1:
                emit_folds(stash.pop(0))
        while stash:
            emit_folds(stash.pop(0))
        while pending_l2:
            _, gf, ibf = pending_l2.pop(0)
            l2fold(gf, ibf)

        # ---------------- finalize ----------------
        with tc.tile_pool(name="fin_sb", bufs=2) as fsb:
            o_sb = accp.tile([128, 2 * OUT], F32, name="o_sb")
            for ib in range(2):
                eng = nc.vector
                u1 = fsb.tile([128, 512], F16, tag="u1")
                eng.tensor_tensor(
                    u1[:], l2buf[ib][:, 0:512], l2buf[ib][:, 512:1024], op=MAX
                )
                u2 = fsb.tile([128, 256], F16, tag="u2")
                eng.tensor_tensor(u2[:], u1[:, 0:256], u1[:, 256:512], op=MAX)
                mraw = fsb.tile([128, MID], F16, tag="mraw")
                eng.tensor_tensor(
                    mraw[:], u2[:, 0:128], u2[:, 128:256], op=MAX
                )
                # msgs^T [mid, i] = max(mraw + cT, -1e6)
                msgs = fsb.tile([128, MID], F32, tag="msgs")
                eng.tensor_tensor(
                    msgs[:], mraw[:], cT_sb[:, ib * MID:(ib + 1) * MID],
                    op=mybir.AluOpType.add,
                )
                eng.tensor_scalar_max(msgs[:], msgs[:], -BIG_NUMBER)
                ps_h_t = ps8p.tile([128, JG * MID], F32, tag="ps8")
                ps_h = ps_h_t[:, 0:OUT]
                nc.tensor.matmul(
                    ps_h, lhsT=msgs[:], rhs=wsb["Wo2"],
                    start=True, stop=True,
                )
                h2s = fsb.tile([128, OUT], F32, tag="h2s")
                nc.vector.tensor_tensor(
                    h2s[:], ps_h, h1pre[:, ib * OUT:(ib + 1) * OUT],
                    op=mybir.AluOpType.add,
                )
                nc.vector.tensor_scalar_max(
                    o_sb[:, ib * OUT:(ib + 1) * OUT], h2s[:], 0.0)
                nc.sync.dma_start(
                    out_d.rearrange("(ib p) o -> p ib o", ib=2)[:, ib],
                    o_sb[:, ib * OUT:(ib + 1) * OUT],
                )

    nc.finalize()
    _assert_safe_pe_schedule(nc)
    return nc


def _assert_safe_pe_schedule(nc):
    """No two adjacent sub-tile (row-grouped) matmuls with different
    tile_position in the final PE stream (HW crash pattern)."""
    prev = None
    for func in nc.m.functions:
        for block in func.blocks:
            for inst in block.instructions:
                if not isinstance(inst, mybir.InstMatmult):
                    continue
                rows = inst.tile_size[0] if inst.tile_size else 128
                sub = rows < 128
                cur = (sub, tuple(inst.tile_position or (0, 0)))
                if (
                    prev is not None
                    and prev[0] and sub
                    and prev[1] != cur[1]
                ):
                    raise AssertionError(
                        f"unsafe adjacent row-grouped matmuls: {prev} -> {cur}"
                    )
                prev = cur
    return True


_CACHED = {}


def _get_program():
    if "nc" not in _CACHED:
        _CACHED["nc"] = _build_program()
    return _CACHED["nc"]


def kernel(**inputs) -> np.ndarray:
    nc = _get_program()

    def f32(x):
        return np.ascontiguousarray(np.asarray(x, dtype=np.float32))

    node_fts = f32(inputs["node_fts"])
    edge_fts = np.asarray(inputs["edge_fts"], dtype=np.float32)
    graph_fts = f32(inputs["graph_fts"])
    adj_mat = np.asarray(inputs["adj_mat"], dtype=np.int32)

    shared = {}
    shared["wpack"] = np.ascontiguousarray(np.concatenate(
        [f32(inputs[w]) for w in ("W2", "W1", "Wg", "Wo1", "Wo2")], axis=1
    ))
    shared["bpack"] = np.ascontiguousarray(np.concatenate(
        [f32(inputs[b]).reshape(1, MID)
         for b in ("b1", "b2", "be", "bg", "bo1", "bo2")], axis=1
    ))
    we = f32(inputs["We"])
    we_hi = we.astype(NP_F8)
    we_lo = (we - we_hi.astype(np.float32)).astype(NP_F8)
    shared["We8"] = np.ascontiguousarray(
        np.concatenate([we_hi, we_lo], axis=1))

    # DoubleRow lhsT template: MASK_NEG in the second 128-block per u on the
    # hi row (even), zeros on the lo row (odd)
    m2t = np.zeros((4, 2, 128, 2, 128), dtype=NP_F8)
    m2t[:, 0, :, 1, :] = np.float32(MASK_NEG).astype(NP_F8)
    shared["m2t"] = np.ascontiguousarray(m2t.reshape(8, 128 * 256))

    in_maps = []
    for c in range(NCORES):
        b, ih = c // 2, c % 2
        sl = slice(ih * IH, (ih + 1) * IH)
        m = dict(shared)
        # edgeT[d, j, i] fp8
        m["edgeT"] = np.ascontiguousarray(
            edge_fts[b, :, sl, :].transpose(2, 0, 1).astype(NP_F8))
        m["nodeT"] = np.ascontiguousarray(node_fts[b].T)
        m["noderT"] = np.ascontiguousarray(node_fts[b, sl, :].T)
        m["graph"] = np.ascontiguousarray(graph_fts[b]).reshape(1, D)
        # adjw[k, u*512 + ib*256 + {0:128 adj01, 128:256 inv01}], j = 128k+u
        a01 = (adj_mat[b, :, sl] != 0)          # [512 j, 256 i]
        aw = np.empty((4, 128, 2, 2, 128), dtype=NP_F8)
        a4 = a01.reshape(4, 128, 2, 128)         # [k, u, ib, i]
        aw[:, :, :, 0, :] = a4.astype(NP_F8)
        aw[:, :, :, 1, :] = (~a4).astype(NP_F8)
        aw = aw.reshape(4, 1, 128 * 512)
        m["adjw"] = np.ascontiguousarray(
            np.broadcast_to(aw, (4, 2, 128 * 512)).reshape(8, 128 * 512))
        in_maps.append(m)

    res = run_bass_kernel_spmd(nc, in_maps, list(range(NCORES)))

    out = np.empty((B, N, OUT), dtype=np.float32)
    for c in range(NCORES):
        b, ih = c // 2, c % 2
        out[b, ih * IH:(ih + 1) * IH, :] = res.results[c]["out"]
    return out


# revision 44
# speedup vs baseline: 1.7352x; 1.0066x over previous
"""Trainium2 Bass kernel for nn_Basic_MPNN (gnn_message_passing).

Math (per batch b):
  m1 = node @ W1 + b1                  [N, MID]   (receiver side, axis 2)
  m2 = node @ W2 + b2                  [N, MID]   (sender side, axis 1)
  me = edge @ We + be                  [N, N, MID]
  mg = graph @ Wg + bg                 [MID]
  msgs[j,i,:] = m1[i] + m2[j] + me[j,i] + mg
  M[i,:] = max_j where(adj[j,i], msgs[j,i,:], -1e6)
  out = relu(node @ Wo1 + bo1 + M @ Wo2 + bo2)

Sharding: 8 cores = (4 batches) x (2 receiver halves of 256).

Host prep (layout/cast only): the edge slice is pre-transposed and
pre-cast to fp8e4m3 as edgeT[d, j, i]; We ships as an fp8 hi/lo pair
(hi = fp8(We), lo = fp8(We - hi)) so the DoubleRow matmul reconstructs
near-f32 weights while streaming fp8; adj ships packed in the DoubleRow
rank-2 rhs layout ([adj01 | inv01] per (j, ib), duplicated on partition
rows 32k and 32k+1, k = j//128).

Per-core device algorithm:
  cT[mid,i] = (m1[i] + mg + b1+be+bg)^T and h1pre = node@Wo1 + bo1+bo2
  computed once (fp32) while the edge stream spins up.
  Per sender-group g (8 j's) one DMA brings the fp8 edgeT tile; each
  (g, ib) unit is a 2-bank PSUM tile (4 in flight). Per (ib, half) a
  DoubleRow matmul (lhsT = [We_hi|We_lo], rhs = the edge tile broadcast
  onto the Ko=2 dim with a stride-0 AP) writes me^T[mid, (jl,i)] at 0.5
  cycles/column; per j a Ki=2 fp8 DoubleRow rank-2 accumulates
    adj01[j,i]*(m2_hi[j,mid]+m2_lo[j,mid]) + inv01[j,i]*MASK_NEG
  (0/1-gated products are exact in fp8; the m2 hi+lo split cancels the
  fp8 quantization of m2 to ~1e-3).
  The 8-slot max reduce (PSUM drain) is split between the only two
  engines that can read PSUM (GpSimd/Pool cannot execute two-tensor ALU
  ops on TRN2 at all): route V (one DVE tensor_reduce straight into the
  acc slot, ~19% of units) and route AV (Act cast-copies each unit into
  half of a shared f16 tile; DVE folds both halves of the pair together
  at 2x). Level-2 f16 folds fire every 8 g's; final fold + cT + clamp +
  one Wo2 matmul + DVE add/relu of h1pre at the end.

Rank-2 row-group placement: all rank-2 matmuls of sender-group g share
PE row-group k = g // 16 (two adjacent row-grouped matmuls with
*different* tile_position inside an open PSUM accumulation group crash
the HW; same tile_position back-to-back is fine, and a full-K matmul
between them is fine). The build asserts the final PE schedule has no
unsafe adjacency.
"""

import os
import sys

for _p in (
    "/root/.axon_site",
    "/root/.axon_site/_ro/trn_rl_repo",
    "/root/.axon_site/_ro/pypackages",
    "/opt/trn_rl_repo",
    "/opt/pypackages",
):
    if os.path.isdir(_p) and _p not in sys.path:
        sys.path.append(_p)

import numpy as np  # noqa: E402

import concourse.bass as bass  # noqa: E402
import concourse.tile as tile  # noqa: E402
from concourse import bacc, mybir  # noqa: E402
from concourse.bass_utils import run_bass_kernel_spmd  # noqa: E402

F32 = mybir.dt.float32
F16 = mybir.dt.float16
F8 = mybir.dt.float8e4
NP_F8 = mybir.dt.np(F8)

B, N, D, MID, OUT = 4, 512, 128, 128, 128
NCORES = 8
IH = N // 2   # receivers per core
JG = 8        # senders per j-group
NG = N // JG  # 64 j-groups
NSLOT = 8     # accumulator slots per (ib, parity)
NWIN = NG // NSLOT  # 8 level-2 windows per ib
IBSHIFT = NSLOT // 2
MASK_NEG = -240.0  # < any valid msg; exact in fp8e4m3 (ml_dtypes e4m3 max is 240)
BIG_NUMBER = 1.0e6
PREFETCH = 4  # edge groups issued ahead of compute

MAX = mybir.AluOpType.max
DR = mybir.MatmulPerfMode.DoubleRow


# Drain route per group g (both ib units take the same route). The
# GpSimd/Pool engine cannot execute any two-tensor ALU op on TRN2 (ISA
# check), so the drain runs entirely on DVE + Act:
#   'V'  = DVE tensor_reduce straight from PSUM, one per unit
#   'AV' = Act casts each unit into half of a shared f16 tile; DVE folds
#          both halves together at 2x once the pair is complete
ROUTE_PATTERN = (
    "V", "AV", "AV", "AV", "AV", "V", "AV", "AV", "AV",
)
ACC_NEG = -60000.0  # f16-representable filler for never-written acc slots


def _build_program():
    nc = bacc.Bacc(
        "TRN2", target_bir_lowering=False, debug=False, num_devices=NCORES
    )

    edgeT = nc.dram_tensor("edgeT", [D, N, IH], F8, kind="ExternalInput").ap()
    nodeT_d = nc.dram_tensor("nodeT", [D, N], F32, kind="ExternalInput").ap()
    noderT_d = nc.dram_tensor("noderT", [D, IH], F32, kind="ExternalInput").ap()
    graph = nc.dram_tensor("graph", [1, D], F32, kind="ExternalInput").ap()
    # DoubleRow rank-2 rhs rows: adjw[k, u*512 + ib*256 + 0:128] = adj01,
    # [...+128:256] = inv01, for j = 128k + u. Loaded onto partitions 32k
    # and 32k+1 (the Ki=2 contraction reads both).
    adjw_d = nc.dram_tensor("adjw", [8, 128 * 512], F8, kind="ExternalInput").ap()
    # DoubleRow lhsT template rows: row 2k (-> partition 32k) has MASK_NEG at
    # u*256+128:256, row 2k+1 (-> partition 32k+1) is zeros; m2_hi/m2_lo are
    # filled into the u*256+0:128 blocks on device.
    m2t_d = nc.dram_tensor("m2t", [8, 128 * 256], F8, kind="ExternalInput").ap()
    wpack_d = nc.dram_tensor("wpack", [D, 5 * MID], F32, kind="ExternalInput").ap()
    bpack_d = nc.dram_tensor("bpack", [1, 6 * MID], F32, kind="ExternalInput").ap()
    # [We_hi | We_lo] fp8 per partition row d
    we8_d = nc.dram_tensor("We8", [D, 2 * MID], F8, kind="ExternalInput").ap()
    out_d = nc.dram_tensor("out", [IH, OUT], F32, kind="ExternalOutput").ap()

    with (
        tile.TileContext(nc) as tc,
        tc.tile_pool(name="persist", bufs=1) as pp,
        tc.tile_pool(name="setup_sb", bufs=1) as ssb,
        tc.tile_pool(name="accum", bufs=1) as accp,
        tc.tile_pool(name="edge", bufs=6) as ep,
        tc.tile_pool(name="tv", bufs=6) as tvp,
        tc.tile_pool(name="ta", bufs=6) as tap,
        tc.tile_pool(name="tf", bufs=6) as tfp,
        tc.tile_pool(name="ps8", bufs=4, space="PSUM") as ps8p,
    ):
        # ---------------- critical-path DMAs first ----------------
        # HWDGE issue slots are ~625 ns each and shared; order matters.
        nodeT = pp.tile([D, N], F32)
        nc.sync.dma_start(nodeT[:], nodeT_d[:, :])
        wpack = pp.tile([D, 5 * MID], F32)
        nc.sync.dma_start(wpack[:, 0:MID], wpack_d[:, 0:MID])  # W2 first
        we8 = pp.tile([D, 2 * MID], F8)
        nc.sync.dma_start(we8[:], we8_d[:, :])

        # ---------------- edge prefetch ----------------
        ej = edgeT.rearrange("d (g j) i -> d g j i", j=JG)
        et_tiles = {}

        def issue_edge(g):
            et = ep.tile([128, JG * IH], F8, tag="e")
            nc.sync.dma_start(et[:], ej[:, g])
            et_tiles[g] = et

        issue_edge(0)

        # rank-2 operands: one 2-partition DMA per row-group k (host ships
        # adj duplicated onto both contraction rows); k=0 ahead of the
        # remaining edge prefetch, later k's and the rest of wpack behind
        adjsb = pp.tile([128, 128 * 512], F8)
        m2dr = pp.tile([128, 128 * 256], F8)
        nc.sync.dma_start(m2dr[0:2, :], m2t_d[0:2, :])
        nc.sync.dma_start(adjsb[0:2, :], adjw_d[0:2, :])
        for g in range(1, PREFETCH):
            issue_edge(g)
        nc.sync.dma_start(wpack[:, MID:], wpack_d[:, MID:])
        for k in range(1, 4):
            nc.sync.dma_start(
                m2dr[32 * k:32 * k + 2, :], m2t_d[2 * k:2 * k + 2, :])
            nc.sync.dma_start(
                adjsb[32 * k:32 * k + 2, :], adjw_d[2 * k:2 * k + 2, :])

        noderT = pp.tile([D, IH], F32)
        nc.gpsimd.dma_start(noderT[:], noderT_d[:, :])
        bpack = pp.tile([1, 6 * MID], F32)
        nc.gpsimd.dma_start(bpack[:], bpack_d[:, :])

        wsb = {
            w: wpack[:, i * MID:(i + 1) * MID]
            for i, w in enumerate(("W2", "W1", "Wg", "Wo1", "Wo2"))
        }
        bsb = {
            b: bpack[:, i * MID:(i + 1) * MID]
            for i, b in enumerate(("b1", "b2", "be", "bg", "bo1", "bo2"))
        }

        ones32 = pp.tile([1, 256], F32)
        nc.vector.memset(ones32[:], 1.0)

        # ---------------- m2 -> fp8 hi/lo DoubleRow lhsT rows ----------------
        # all four k-blocks in one borrowed PSUM tile, one wide cast chain
        ps_m2_t = ps8p.tile([128, JG * MID], F32, tag="ps8")
        for k in range(4):
            # b2 is sender-independent, so it is folded into r (the max
            # commutes with adding a j-constant); m2 here is node @ W2 only
            ps_k = ps_m2_t[:, k * MID:(k + 1) * MID]
            nc.tensor.matmul(
                ps_k,
                lhsT=nodeT[:, k * 128:(k + 1) * 128],
                rhs=wsb["W2"], start=True, stop=True,
            )
        m2f32 = ssb.tile([128, 4 * MID], F32)
        m2h8 = ssb.tile([128, 4 * MID], F8)
        m2hup = ssb.tile([128, 4 * MID], F32)
        m2l8 = ssb.tile([128, 4 * MID], F8)

        def m2_chain(sl):
            nc.scalar.copy(m2f32[:, sl], ps_m2_t[:, sl])
            nc.vector.tensor_copy(m2h8[:, sl], m2f32[:, sl])
            nc.vector.tensor_copy(m2hup[:, sl], m2h8[:, sl])
            nc.vector.tensor_tensor(
                m2l8[:, sl], m2f32[:, sl], m2hup[:, sl],
                op=mybir.AluOpType.subtract,
            )

        def m2_place(k):
            # m2dr[32k, u*256 + 0:128] = m2_hi[128k+u, :]; 32k+1 gets m2_lo
            for row, srct in ((0, m2h8), (1, m2l8)):
                dst = m2dr[32 * k + row:32 * k + row + 1, :].rearrange(
                    "p (u c) -> p u c", c=256
                )[:, :, 0:128]
                nc.scalar.dma_start(dst, srct[:, k * MID:(k + 1) * MID])

        m2_chain(slice(0, MID))          # k=0 first: unblocks group 0
        m2_place(0)
        m2_chain(slice(MID, 4 * MID))
        for k in range(1, 4):
            m2_place(k)

        # r = mg + b1 + be + bg ; bso = bo1 + bo2 ; cT[mid,i] = (m1 + r)^T
        gT = ssb.tile([D, 1], F32)
        nc.gpsimd.dma_start(gT[:], graph[0:1, :])
        ps_fin_t = ps8p.tile([128, JG * MID], F32, tag="ps8")
        ps_mg = ps_fin_t[0:1, 0:MID]
        nc.tensor.matmul(ps_mg, lhsT=gT[:], rhs=wsb["Wg"], start=True, stop=True)
        r_sb = pp.tile([1, MID], F32)
        nc.scalar.copy(r_sb[:], ps_mg)
        nc.vector.tensor_add(r_sb[:], r_sb[:], bsb["b1"])
        nc.vector.tensor_add(r_sb[:], r_sb[:], bsb["b2"])
        nc.vector.tensor_add(r_sb[:], r_sb[:], bsb["be"])
        nc.vector.tensor_add(r_sb[:], r_sb[:], bsb["bg"])
        bso = pp.tile([1, MID], F32)
        nc.vector.tensor_add(bso[:], bsb["bo1"], bsb["bo2"])

        ps_cT = ps_fin_t[:, MID:MID + IH]
        nc.tensor.matmul(
            ps_cT, lhsT=wsb["W1"][:], rhs=noderT[:], start=True, stop=False
        )
        nc.tensor.matmul(
            ps_cT, lhsT=r_sb[:], rhs=ones32[:], start=False, stop=True
        )
        cT_sb = pp.tile([128, IH], F32)
        nc.scalar.copy(cT_sb[:], ps_cT)

        # h1pre[i, out] = node @ Wo1 + bo1 + bo2 (receiver slice), so the
        # finalize needs only the msgs @ Wo2 matmul
        h1pre = pp.tile([128, 2 * OUT], F32)
        ps_h1_t = ps8p.tile([128, JG * MID], F32, tag="ps8")
        for ib in range(2):
            ps_h1 = ps_h1_t[:, ib * 512:ib * 512 + OUT]
            nc.tensor.matmul(
                ps_h1, lhsT=noderT[:, ib * 128:(ib + 1) * 128],
                rhs=wsb["Wo1"], start=True, stop=False,
            )
            nc.tensor.matmul(
                ps_h1, lhsT=ones32[:, 0:128], rhs=bso[:],
                start=False, stop=True,
            )
            nc.scalar.copy(h1pre[:, ib * OUT:(ib + 1) * OUT], ps_h1)

        # ---------------- accumulators ----------------
        # acc: [128, (ib 2, parity 2, slot 8, 128)] f16
        acc = accp.tile([128, 2 * 2 * NSLOT * 128], F16, name="acc")
        accv = acc[:].rearrange("p (i c) -> p i c", i=2)
        l2buf = [accp.tile([128, NWIN * 128], F16, name=f"l2{ib}")
                 for ib in range(2)]
        l2pre = accp.tile([128, 2 * 128], F16, name="l2pre")

        def acc_dst(g, ib):
            off = ((g // NSLOT) % 2 * NSLOT + g % NSLOT) * 128
            return acc[:, ib * 2048 + off:ib * 2048 + off + 128]

        def acc_dst2(g):
            # [p, 2, 128] strided AP over both ibs' slots for group g
            off = ((g // NSLOT) % 2 * NSLOT + g % NSLOT) * 128
            return accv[:, :, off:off + 128]

        def emit_release(ps8, g, ib, route, c1):
            """The op that frees the PSUM unit: a single DVE reduce (V), or
            an Act cast copy into half of the pair's shared f16 tile."""
            if route == "V":
                nc.vector.tensor_reduce(
                    acc_dst(g, ib),
                    ps8.rearrange("p (s m) -> p m s", s=JG),
                    axis=mybir.AxisListType.X,
                    op=MAX,
                )
            else:
                nc.scalar.copy(c1[:, ib * 1024:(ib + 1) * 1024], ps8)

        def emit_folds(st):
            g, route, c1 = st
            if route != "V":
                tv = c1[:].rearrange("p (i c) -> p i c", i=2)
                t1 = tvp.tile([128, 1024], F16, tag="tv")
                t1v = t1[:].rearrange("p (i c) -> p i c", i=2)
                nc.vector.tensor_tensor(
                    t1v, tv[:, :, 0:512], tv[:, :, 512:1024], op=MAX)
                t2 = tfp.tile([128, 512], F16, tag="tf")
                t2v = t2[:].rearrange("p (i c) -> p i c", i=2)
                nc.vector.tensor_tensor(
                    t2v, t1v[:, :, 0:256], t1v[:, :, 256:512], op=MAX)
                nc.vector.tensor_tensor(
                    acc_dst2(g), t2v[:, :, 0:128], t2v[:, :, 128:256], op=MAX)
            if g % NSLOT == NSLOT - 1:
                for ib in range(2):
                    l2fold(g, ib)

        def l2fold(g, ib):
            # fold the just-completed parity's NSLOT slots -> l2buf slot
            par = (g // NSLOT) % 2
            base = (ib * 2 + par) * NSLOT * 128
            src = acc[:, base:base + NSLOT * 128]
            t1 = tvp.tile([128, 512], F16, tag="tvl")
            nc.vector.tensor_tensor(t1[:], src[:, 0:512], src[:, 512:1024],
                                    op=MAX)
            t3 = tfp.tile([128, 256], F16, tag="tf")
            nc.vector.tensor_tensor(t3[:], t1[:, 0:256], t1[:, 256:512],
                                    op=MAX)
            lslot = g // NSLOT
            nc.vector.tensor_tensor(
                l2buf[ib][:, lslot * 128:(lslot + 1) * 128],
                t3[:, 0:128], t3[:, 128:256], op=MAX,
            )

        # ---------------- main streaming loop ----------------
        stash = []
        for g in range(NG):
            if g + PREFETCH < NG:
                issue_edge(g + PREFETCH)
            k = (g * JG) // 128
            et = et_tiles.pop(g)
            etv = et[:].rearrange("p (j i) -> p j i", j=JG)
            route = "V" if g == NG - 1 else ROUTE_PATTERN[g % len(ROUTE_PATTERN)]
            c1 = None
            if route != "V":
                c1 = tap.tile([128, 2048], F16, tag="ta")
            for ib in range(2):
                ps8_t = ps8p.tile([128, JG * MID], F32, tag="ps8")
                ps8 = ps8_t[:]
                for half in range(2):
                    rhs = (
                        etv[:, 4 * half:4 * half + 4, ib * 128:(ib + 1) * 128]
                        .unsqueeze(1)
                        .broadcast_to([128, 2, 4, 128])
                    )
                    nc.tensor.matmul(
                        ps8[:, half * 512:(half + 1) * 512],
                        lhsT=we8[:].rearrange("p (two f) -> p two f", two=2),
                        rhs=rhs,
                        start=True, stop=False,
                        perf_mode=DR,
                    )
                    for q in range(4):
                        jl = half * 4 + q
                        u = (g * JG + jl) % 128
                        lhsT = m2dr[32 * k:32 * k + 2,
                                    u * 256:(u + 1) * 256].rearrange(
                            "p (two f) -> p two f", two=2)
                        rhs2 = adjsb[32 * k:32 * k + 2,
                                     u * 512 + ib * 256:
                                     u * 512 + ib * 256 + 256].rearrange(
                            "p (two f) -> p two f", two=2)
                        nc.tensor.matmul(
                            ps8[:, jl * MID:(jl + 1) * MID],
                            lhsT=lhsT, rhs=rhs2,
                            start=False, stop=(q == 3),
                            perf_mode=DR,
                            tile_position=(32 * k, 0),
                        )
                emit_release(ps8, g, ib, route, c1)
            stash.append((g, route, c1))
            if len(stash) > 1:
                emit_folds(stash.pop(0))
            if g == 44:
                for ib in range(2):
                    pa = tfp.tile([128, 256], F16, tag="tf")
                    nc.vector.tensor_tensor(
                        pa[:], l2buf[ib][:, 0:256], l2buf[ib][:, 256:512],
                        op=MAX)
                    nc.vector.tensor_tensor(
                        l2pre[:, ib * 128:(ib + 1) * 128],
                        pa[:, 0:128], pa[:, 128:256], op=MAX)
        while stash:
            emit_folds(stash.pop(0))

        # ---------------- finalize ----------------
        with tc.tile_pool(name="fin_sb", bufs=2) as fsb:
            o_sb = accp.tile([128, 2 * OUT], F32, name="o_sb")
            for ib in range(2):
                eng = nc.vector
                u1 = fsb.tile([128, 256], F16, tag="u1")
                eng.tensor_tensor(
                    u1[:], l2buf[ib][:, 512:768], l2buf[ib][:, 768:1024],
                    op=MAX)
                u2 = fsb.tile([128, MID], F16, tag="u2")
                eng.tensor_tensor(u2[:], u1[:, 0:128], u1[:, 128:256], op=MAX)
                mraw = fsb.tile([128, MID], F16, tag="mraw")
                eng.tensor_tensor(
                    mraw[:], u2[:], l2pre[:, ib * 128:(ib + 1) * 128], op=MAX
                )
                # msgs^T [mid, i] = max(mraw + cT, -1e6)
                msgs = fsb.tile([128, MID], F32, tag="msgs")
                eng.tensor_tensor(
                    msgs[:], mraw[:], cT_sb[:, ib * MID:(ib + 1) * MID],
                    op=mybir.AluOpType.add,
                )
                eng.tensor_scalar_max(msgs[:], msgs[:], -BIG_NUMBER)
                ps_h_t = ps8p.tile([128, JG * MID], F32, tag="ps8")
                ps_h = ps_h_t[:, 0:OUT]
                nc.tensor.matmul(
                    ps_h, lhsT=msgs[:], rhs=wsb["Wo2"],
                    start=True, stop=True,
                )
                h2s = fsb.tile([128, OUT], F32, tag="h2s")
                nc.vector.tensor_tensor(
                    h2s[:], ps_h, h1pre[:, ib * OUT:(ib + 1) * OUT],
                    op=mybir.AluOpType.add,
                )
                nc.vector.tensor_scalar_max(
                    o_sb[:, ib * OUT:(ib + 1) * OUT], h2s[:], 0.0)
                nc.sync.dma_start(
                    out_d.rearrange("(ib p) o -> p ib o", ib=2)[:, ib],
                    o_sb[:, ib * OUT:(ib + 1) * OUT],
                )

    nc.finalize()
    _assert_safe_pe_schedule(nc)
    return nc


def _assert_safe_pe_schedule(nc):
    """No two adjacent sub-tile (row-grouped) matmuls with different
    tile_position in the final PE stream (HW crash pattern)."""
    prev = None
    for func in nc.m.functions:
        for block in func.blocks:
            for inst in block.instructions:
                if not isinstance(inst, mybir.InstMatmult):
                    continue
                rows = inst.tile_size[0] if inst.tile_size else 128
                sub = rows < 128
                cur = (sub, tuple(inst.tile_position or (0, 0)))
                if (
                    prev is not None
                    and prev[0] and sub
                    and prev[1] != cur[1]
                ):
                    raise AssertionError(
                        f"unsafe adjacent row-grouped matmuls: {prev} -> {cur}"
                    )
                prev = cur
    return True


_CACHED = {}


def _get_program():
    if "nc" not in _CACHED:
        _CACHED["nc"] = _build_program()
    return _CACHED["nc"]


def kernel(**inputs) -> np.ndarray:
    nc = _get_program()

    def f32(x):
        return np.ascontiguousarray(np.asarray(x, dtype=np.float32))

    node_fts = f32(inputs["node_fts"])
    edge_fts = np.asarray(inputs["edge_fts"], dtype=np.float32)
    graph_fts = f32(inputs["graph_fts"])
    adj_mat = np.asarray(inputs["adj_mat"], dtype=np.int32)

    shared = {}
    shared["wpack"] = np.ascontiguousarray(np.concatenate(
        [f32(inputs[w]) for w in ("W2", "W1", "Wg", "Wo1", "Wo2")], axis=1
    ))
    shared["bpack"] = np.ascontiguousarray(np.concatenate(
        [f32(inputs[b]).reshape(1, MID)
         for b in ("b1", "b2", "be", "bg", "bo1", "bo2")], axis=1
    ))
    we = f32(inputs["We"])
    we_hi = we.astype(NP_F8)
    we_lo = (we - we_hi.astype(np.float32)).astype(NP_F8)
    shared["We8"] = np.ascontiguousarray(
        np.concatenate([we_hi, we_lo], axis=1))

    # DoubleRow lhsT template: MASK_NEG in the second 128-block per u on the
    # hi row (even), zeros on the lo row (odd)
    m2t = np.zeros((4, 2, 128, 2, 128), dtype=NP_F8)
    m2t[:, 0, :, 1, :] = np.float32(MASK_NEG).astype(NP_F8)
    shared["m2t"] = np.ascontiguousarray(m2t.reshape(8, 128 * 256))

    in_maps = []
    for c in range(NCORES):
        b, ih = c // 2, c % 2
        sl = slice(ih * IH, (ih + 1) * IH)
        m = dict(shared)
        # edgeT[d, j, i] fp8
        m["edgeT"] = np.ascontiguousarray(
            edge_fts[b, :, sl, :].transpose(2, 0, 1).astype(NP_F8))
        m["nodeT"] = np.ascontiguousarray(node_fts[b].T)
        m["noderT"] = np.ascontiguousarray(node_fts[b, sl, :].T)
        m["graph"] = np.ascontiguousarray(graph_fts[b]).reshape(1, D)
        # adjw[k, u*512 + ib*256 + {0:128 adj01, 128:256 inv01}], j = 128k+u
        a01 = (adj_mat[b, :, sl] != 0)          # [512 j, 256 i]
        aw = np.empty((4, 128, 2, 2, 128), dtype=NP_F8)
        a4 = a01.reshape(4, 128, 2, 128)         # [k, u, ib, i]
        aw[:, :, :, 0, :] = a4.astype(NP_F8)
        aw[:, :, :, 1, :] = (~a4).astype(NP_F8)
        aw = aw.reshape(4, 1, 128 * 512)
        m["adjw"] = np.ascontiguousarray(
            np.broadcast_to(aw, (4, 2, 128 * 512)).reshape(8, 128 * 512))
        in_maps.append(m)

    res = run_bass_kernel_spmd(nc, in_maps, list(range(NCORES)))

    out = np.empty((B, N, OUT), dtype=np.float32)
    for c in range(NCORES):
        b, ih = c // 2, c % 2
        out[b, ih * IH:(ih + 1) * IH, :] = res.results[c]["out"]
    return out
